# revision 1
# baseline (speedup 1.0000x reference)
"""ChebyNet (K=5, 7 ChebConv layers) on 8 trn2 NeuronCores via Bass/Tile.

Strategy (matches the sharding hint):
- Nodes are sharded across the 8 cores (snake-dealt by in-degree, then
  degree-sorted within each core so padded-CSR tiles have uniform degree).
- Edges are partitioned by destination-node owner; each Chebyshev step
  AllGathers the scaled source features (fp32 table in DRAM) and each core
  gathers its in-edges with dma_gather (int16 indices -> 4 source-range
  passes), then segment-sums on-chip with strided DVE reduces.
- The per-edge norm -dinv[src]*dinv[dst] is factorized: the gathered table
  holds dinv*Tx, and -dinv (folded with the Chebyshev 2x) is applied to the
  reduced sums per destination.
- conv0 (128->64) is evaluated with Clenshaw recurrence so all gather steps
  run at feature dim 64: b_k = x@W0_k + 2*A b_{k+1} - b_{k+2}.
- The dense out = sum_k Tx_k @ W_k for convs 1..6 accumulates feature-major
  via an fp16 DMA-transpose readback of each Tx_k and PE matmuls.
"""

import math

import numpy as np

from concourse import bacc, bass, mybir, tile
from concourse.bass_interp import get_hw_module
from concourse.bass_utils import run_bass_kernel_spmd
from concourse.masks import make_identity

P = 128
NCORES = 8
BN_EPS = 1e-5
OUT_DIM = 10

F32 = mybir.dt.float32
F16 = mybir.dt.float16
I16 = mybir.dt.int16
I32 = mybir.dt.int32
Alu = mybir.AluOpType
Axis = mybir.AxisListType
Act = mybir.ActivationFunctionType


# ----------------------------------------------------------------------------
# host-side preprocessing
# ----------------------------------------------------------------------------

def host_prep(x, edge_index, batch, conv0_W, conv0_b, conv1_W, conv1_b,
              bn_gamma, bn_beta, bn_mean, bn_var, lin2_W, lin2_b,
              num_graphs, num_step_prop, verbose=False):
    N, IN_DIM = x.shape
    K, _, HID = np.asarray(conv0_W).shape
    row = np.asarray(edge_index[0], dtype=np.int64)
    col = np.asarray(edge_index[1], dtype=np.int64)
    batch = np.asarray(batch, dtype=np.int64)
    E = row.shape[0]

    deg = np.bincount(row, minlength=N).astype(np.float64)
    dinv = np.where(deg > 0, 1.0 / np.sqrt(np.maximum(deg, 1.0)), 0.0)
    dinv = dinv.astype(np.float32)

    indeg = np.bincount(col, minlength=N)

    # --- shard nodes: snake-deal by in-degree; per-core sort by in-degree ---
    order = np.argsort(-indeg, kind="stable")
    core_of = np.empty(N, dtype=np.int64)
    idxs_all = np.arange(N)
    r, c = idxs_all // NCORES, idxs_all % NCORES
    snake = np.where(r % 2 == 0, c, NCORES - 1 - c)
    core_of[order] = snake

    TPC = math.ceil(N / NCORES / P)
    if N >= NCORES * TPC * P:
        TPC += 1  # always keep at least one pad slot per core (zero rows)
    SPC = TPC * P
    NALL = NCORES * SPC
    QR = NALL // 4          # balanced int16 range size (== 2*SPC)
    assert QR <= 32768 and QR == 2 * SPC

    # per-core slot assignment (slot s in [0, SPC); pads at the end)
    node_of_slot = np.full((NCORES, SPC), -1, dtype=np.int64)
    slot_of_node = np.empty(N, dtype=np.int64)
    core_slot_count = np.zeros(NCORES, dtype=np.int64)
    for cc in range(NCORES):
        nodes = np.where(core_of == cc)[0]
        nodes = nodes[np.argsort(-indeg[nodes], kind="stable")]
        assert len(nodes) <= SPC - 1, "need at least one pad slot"
        node_of_slot[cc, :len(nodes)] = nodes
        slot_of_node[nodes] = np.arange(len(nodes))
        core_slot_count[cc] = len(nodes)

    # slot -> local table row (pair interleave within each 128-slot tile)
    sl = np.arange(SPC)
    # tile 2u holds even rows, tile 2u+1 odd rows of group [256u, 256u+256)
    l_of_slot = 256 * (sl // P // 2) + 2 * (sl % P) + ((sl // P) % 2)
    row_of_node = np.empty(N, dtype=np.int64)
    for cc in range(NCORES):
        nn = core_slot_count[cc]
        nodes = node_of_slot[cc, :nn]
        row_of_node[nodes] = cc * SPC + l_of_slot[:nn]

    # zero rows (pad slots) per range: quarter q holds cores 2q, 2q+1 exactly
    zrow = np.empty(4, dtype=np.int64)
    for q in range(4):
        cc = 2 * q
        pad_slot = core_slot_count[cc]
        assert pad_slot < SPC
        zr = cc * SPC + l_of_slot[pad_slot]
        assert q * QR <= zr < (q + 1) * QR
        zrow[q] = zr

    # --- edges bucketed by (dest core, dest slot, source range) ---
    src_row = row_of_node[row]
    src_range = src_row // QR
    dst_core = core_of[col]
    dst_slot = slot_of_node[col]

    cnt = np.zeros((NCORES, SPC, 4), dtype=np.int32)
    np.add.at(cnt, (dst_core, dst_slot, src_range), 1)

    ekey = (dst_core * SPC + dst_slot) * 4 + src_range
    eorder = np.argsort(ekey, kind="stable")
    srcs_sorted = src_row[eorder]
    key_sorted = ekey[eorder]
    starts = np.searchsorted(key_sorted, np.arange(NCORES * SPC * 4))

    # chunk layout (tiles of 128 slots; chunks of up to 8 tiles)
    chunk_tiles = []
    t = 0
    while t < TPC:
        nt = min(8, TPC - t)
        chunk_tiles.append((t, nt))
        t += nt

    # single-pass padded CSR: per-chunk uniform L, int32 global rows
    cnt1 = cnt.sum(axis=2)  # [NCORES, SPC]
    chunks_meta = []
    col_off = 0
    for (t0, nt) in chunk_tiles:
        L = int(cnt1[:, t0 * P:(t0 + nt) * P].max())
        L = max(L, 1)
        chunks_meta.append({"L": L, "off": col_off})
        col_off += nt * L
    idx_width = col_off

    total_padded = sum(
        nt * pm["L"] * P for pm, (t0, nt) in zip(chunks_meta, chunk_tiles))
    if verbose:
        print(f"[prep] TPC={TPC} SPC={SPC} NALL={NALL} "
              f"padded/E per core = {total_padded / (E / NCORES):.3f} "
              f"idx_width={idx_width}")

    # per-(core, slot) flat edge lists (all ranges concatenated)
    idx_arrays = []
    for cc in range(NCORES):
        arr = np.full((P, idx_width), zrow[0], dtype=np.int32)
        for ci, (t0, nt) in enumerate(chunk_tiles):
            L = chunks_meta[ci]["L"]
            off = chunks_meta[ci]["off"]
            ns = nt * P
            s0 = t0 * P
            base_keys = (cc * SPC + np.arange(s0, s0 + ns)) * 4
            cnts = cnt1[cc, s0:s0 + ns].astype(np.int64)
            tot = int(cnts.sum())
            if tot == 0:
                continue
            slot_rep = np.repeat(np.arange(ns), cnts)
            rank = (np.arange(tot) -
                    np.repeat(np.cumsum(cnts) - cnts, cnts))
            # per-slot edges = concat of its 4 range buckets in the sorted
            # array; they are contiguous since key = slot*4 + q
            offs0 = starts[base_keys]
            eidx = np.repeat(offs0, cnts) + rank
            tt = slot_rep // P
            p = slot_rep % P
            cols = off + tt * L + rank
            arr[p, cols] = srcs_sorted[eidx]
        idx_arrays.append(arr)

    # --- dense per-core inputs ---
    gscale = (np.asarray(bn_gamma, np.float64) /
              np.sqrt(np.asarray(bn_var, np.float64) + BN_EPS))
    gbias = (np.asarray(bn_beta, np.float64) -
             np.asarray(bn_mean, np.float64) * gscale)
    gscale = gscale.astype(np.float32)
    gbias = gbias.astype(np.float32)
    b0 = np.asarray(conv0_b, np.float32)
    b1 = np.asarray(conv1_b, np.float32)

    counts = np.bincount(batch, minlength=num_graphs).astype(np.float64)
    invcnt = np.zeros((64, 1), dtype=np.float32)
    invcnt[:num_graphs, 0] = (1.0 / np.maximum(counts, 1.0)).astype(np.float32)

    W0all = np.ascontiguousarray(
        np.asarray(conv0_W, np.float32).transpose(1, 0, 2).reshape(
            IN_DIM, K * HID))
    W1 = np.asarray(conv1_W, np.float32)
    W1pack = np.zeros((P, K * HID), dtype=np.float16)
    for k in range(K):
        W1pack[:64, k * HID:(k + 1) * HID] = W1[k].astype(np.float16)
        W1pack[64:, k * HID:(k + 1) * HID] = W1[k].astype(np.float16)

    def packed64(v):
        out = np.empty((P, 1), dtype=np.float32)
        out[:64, 0] = v
        out[64:, 0] = v
        return out

    def bcast128(v):
        return np.tile(np.asarray(v, np.float32)[None, :], (P, 1))

    x = np.asarray(x, np.float32)
    in_maps = []
    for cc in range(NCORES):
        nn = core_slot_count[cc]
        nodes = node_of_slot[cc, :nn]
        s = np.arange(nn)
        tt, pp = s // P, s % P
        xT = np.zeros((IN_DIM, SPC), dtype=np.float32)
        xT[:, :nn] = x[nodes].T
        dinv_slot = np.zeros((P, TPC), dtype=np.float32)
        dinv_slot[pp, tt] = dinv[nodes]
        onehot = np.zeros((P, TPC * 64), dtype=np.float32)
        gg = batch[nodes]
        m = gg < 64
        onehot[pp[m], tt[m] * 64 + gg[m]] = 1.0
        in_maps.append({
            "xT": xT,
            "idxs": idx_arrays[cc],
            "dinv_slot": dinv_slot,
            "onehot": onehot,
            "W0all": W0all,
            "W1pack": W1pack,
            "b0bc": bcast128(b0),
            "gsbc": bcast128(gscale),
            "gbbc": bcast128(gbias),
            "b1p": packed64(b1),
            "gsp": packed64(gscale),
            "gbp": packed64(gbias),
            "invcnt": invcnt,
            "W2": np.asarray(lin2_W, np.float32),
            "b2bc": np.tile(np.asarray(lin2_b, np.float32)[None, :], (64, 1)),
        })

    meta = {
        "N": N, "IN_DIM": IN_DIM, "HID": HID, "K": K, "TPC": TPC,
        "SPC": SPC, "NALL": NALL, "QR": QR, "NG": num_graphs,
        "NPROP": num_step_prop, "chunk_tiles": chunk_tiles,
        "chunks_meta": chunks_meta, "idx_width": idx_width,
        "node_of_slot": node_of_slot,
    }
    return in_maps, meta


# ----------------------------------------------------------------------------
# kernel builder
# ----------------------------------------------------------------------------

def _mid_bcast(ap, n):
    """[P, D] AP -> [P, n, D] AP with a stride-0 middle dim."""
    return bass.AP(ap.tensor, ap.offset, [ap.ap[0], [0, n], ap.ap[1]])


def build_kernel(meta, debug_stop=None):
    TPC, SPC, NALL, QR = meta["TPC"], meta["SPC"], meta["NALL"], meta["QR"]
    K, HID, IN_DIM = meta["K"], meta["HID"], meta["IN_DIM"]
    NPROP = meta["NPROP"]
    chunk_tiles = meta["chunk_tiles"]
    chunks_meta = meta["chunks_meta"]

    nc = bacc.Bacc("TRN2", target_bir_lowering=False, debug=False,
                   num_devices=NCORES)

    # ---- I/O ----
    xT_d = nc.dram_tensor("xT", [IN_DIM, SPC], F32, kind="ExternalInput")
    idxs_d = nc.dram_tensor("idxs", [P, meta["idx_width"]], I32,
                            kind="ExternalInput")
    dinv_d = nc.dram_tensor("dinv_slot", [P, TPC], F32, kind="ExternalInput")
    oneh_d = nc.dram_tensor("onehot", [P, TPC * 64], F32, kind="ExternalInput")
    W0_d = nc.dram_tensor("W0all", [IN_DIM, K * HID], F32,
                          kind="ExternalInput")
    W1_d = nc.dram_tensor("W1pack", [P, K * HID], F16, kind="ExternalInput")
    b0bc_d = nc.dram_tensor("b0bc", [P, HID], F32, kind="ExternalInput")
    gsbc_d = nc.dram_tensor("gsbc", [P, HID], F32, kind="ExternalInput")
    gbbc_d = nc.dram_tensor("gbbc", [P, HID], F32, kind="ExternalInput")
    b1p_d = nc.dram_tensor("b1p", [P, 1], F32, kind="ExternalInput")
    gsp_d = nc.dram_tensor("gsp", [P, 1], F32, kind="ExternalInput")
    gbp_d = nc.dram_tensor("gbp", [P, 1], F32, kind="ExternalInput")
    invc_d = nc.dram_tensor("invcnt", [64, 1], F32, kind="ExternalInput")
    W2_d = nc.dram_tensor("W2", [HID, OUT_DIM], F32, kind="ExternalInput")
    b2bc_d = nc.dram_tensor("b2bc", [64, OUT_DIM], F32, kind="ExternalInput")
    out_d = nc.dram_tensor("out", [64, OUT_DIM], F32, kind="ExternalOutput")
    dbg_d = nc.dram_tensor("dbg", [P, TPC * HID], F32, kind="ExternalOutput")

    # ---- internal DRAM ----
    tables = [nc.dram_tensor(f"table{i}", [NALL, HID], F32,
                             addr_space="Shared") for i in range(2)]
    tablesL = [nc.dram_tensor(f"tableL{i}", [NALL, HID], F32)
               for i in range(2)]
    cins = [nc.dram_tensor(f"cin{i}", [SPC, HID], F32) for i in range(2)]
    craws = [nc.dram_tensor(f"craw{i}", [SPC, HID], F16) for i in range(2)]
    tall0 = nc.dram_tensor("tall0", [SPC, K * HID], F32)
    pool_in = nc.dram_tensor("pool_in", [64, HID], F32)
    pool_out = nc.dram_tensor("pool_out", [64, HID], F32, addr_space="Shared")

    def lrow_ap(dram, t0, nt, b):
        """DRAM AP for rows l = 256*(t//2) + 2*p + b over parity-b tiles of
        the chunk: matches SBUF [128, nt//2, HID]."""
        return bass.AP(dram.ap().tensor, t0 * P * HID + b * HID,
                       [[2 * HID, P], [2 * P * HID, nt // 2], [1, HID]])

    def parity_view(ap2d, nt, b):
        """[128, nt*HID] contiguous-free AP -> [128, nt//2, HID] tiles of
        parity b."""
        return bass.AP(ap2d.tensor, ap2d.offset + b * HID,
                       [ap2d.ap[0], [2 * HID, nt // 2], [1, HID]])

    with tile.TileContext(nc) as tc:
        with (
            tc.tile_pool(name="state", bufs=1) as st,
            tc.tile_pool(name="consts", bufs=1) as cp,
            tc.tile_pool(name="gbuf", bufs=2) as gp,
            tc.tile_pool(name="small", bufs=2) as sp,
            tc.tile_pool(name="xtp", bufs=2) as xp,
            tc.tile_pool(name="idxp", bufs=2) as ip,
            tc.tile_pool(name="rbp", bufs=2) as rp,
            tc.tile_pool(name="psA", bufs=2, space="PSUM") as psA,
            tc.tile_pool(name="psB", bufs=2, space="PSUM") as psB,
            tc.tile_pool(name="psC", bufs=1, space="PSUM") as psC,
        ):
            stA = st.tile([P, TPC * HID], F32, tag="stA")
            stB = st.tile([P, TPC * HID], F32, tag="stB")
            acc = st.tile([P, SPC // 2], F32, tag="acc")

            dinv_t = cp.tile([P, TPC], F32, tag="dinv")
            nc.sync.dma_start(out=dinv_t[:], in_=dinv_d[:])
            W0_t = cp.tile([IN_DIM, K * HID], F32, tag="w0")
            nc.sync.dma_start(out=W0_t[:], in_=W0_d[:])
            W1_t = cp.tile([P, K * HID], F16, tag="w1")
            nc.sync.dma_start(out=W1_t[:], in_=W1_d[:])
            b0bc = cp.tile([P, HID], F32, tag="b0bc")
            nc.sync.dma_start(out=b0bc[:], in_=b0bc_d[:])
            gsbc = cp.tile([P, HID], F32, tag="gsbc")
            nc.sync.dma_start(out=gsbc[:], in_=gsbc_d[:])
            gbbc = cp.tile([P, HID], F32, tag="gbbc")
            nc.sync.dma_start(out=gbbc[:], in_=gbbc_d[:])
            b1p = cp.tile([P, 1], F32, tag="b1p")
            nc.sync.dma_start(out=b1p[:], in_=b1p_d[:])
            gsp = cp.tile([P, 1], F32, tag="gsp")
            nc.sync.dma_start(out=gsp[:], in_=gsp_d[:])
            gbp = cp.tile([P, 1], F32, tag="gbp")
            nc.sync.dma_start(out=gbp[:], in_=gbp_d[:])
            iden = cp.tile([P, P], F32, tag="iden")
            make_identity(nc, iden[:])

            def dinv_bc(t0, nt):
                return dinv_t[:, t0:t0 + nt].to_broadcast([P, nt, HID])

            def st3(ap):
                return ap.rearrange("p (t d) -> p t d", d=HID)

            # ---- conv0 projections: tall0[s, k*64+f] = x[s] @ W0[k] ----
            for (t0, nt) in chunk_tiles:
                xTc = xp.tile([IN_DIM, nt * P], F32, tag="xTc")
                nc.sync.dma_start(out=xTc[:, :nt * P],
                                  in_=xT_d[:, t0 * P:(t0 + nt) * P])
                for tt in range(nt):
                    pm = psA.tile([P, 512], F32, space="PSUM", tag="ptr")
                    nc.tensor.matmul(pm[:, :K * HID],
                                     lhsT=xTc[:, tt * P:(tt + 1) * P],
                                     rhs=W0_t[:], start=True, stop=True)
                    pj = sp.tile([P, K * HID], F32, tag="projsb")
                    nc.vector.tensor_copy(pj[:], pm[:, :K * HID])
                    nc.sync.dma_start(
                        out=tall0.ap()[(t0 + tt) * P:(t0 + tt + 1) * P, :],
                        in_=pj[:])

            # ---- load B4 = c4 into stA; write cin0 = dinv * B4 ----
            for (t0, nt) in chunk_tiles:
                stc = stA[:, t0 * HID:(t0 + nt) * HID]
                nc.sync.dma_start(
                    out=st3(stc),
                    in_=tall0.ap()[t0 * P:(t0 + nt) * P,
                                   (K - 1) * HID:K * HID]
                        .rearrange("(t p) d -> p t d", p=P))
                cst = sp.tile([P, nt * HID], F32, tag="cinst")
                nc.vector.tensor_tensor(out=st3(cst[:, :nt * HID]),
                                        in0=st3(stc), in1=dinv_bc(t0, nt),
                                        op=Alu.mult)
                for _b in (0, 1):
                                      nc.sync.dma_start(
                                          out=lrow_ap(cins[0], t0, nt, _b),
                                          in_=parity_view(cst[:, :nt * HID], nt, _b))

            state = {"step": 0, "go": True}

            def stop_after(name, buf):
                if state["go"] and debug_stop == name:
                    nc.sync.dma_start(out=dbg_d[:], in_=buf[:])
                    state["go"] = False

            def gather_step(into_state, mode, cj=None,
                            write_cin=True, write_craw=False):
                """One Lhat application. Modes:
                clenshaw_first: into = -2m + c_j
                clenshaw:       into = (-2m + c_j) - into_old
                clenshaw_last:  into = (-m + c_j) - into_old
                fwd_first:      into = -m
                fwd:            into = -2m - into_old
                """
                s = state["step"]
                par, nxt = s % 2, (s + 1) % 2
                tbl = tables[par]
                nc.gpsimd.collective_compute(
                    "AllGather", Alu.bypass,
                    replica_groups=[list(range(NCORES))],
                    ins=[cins[par][:]], outs=[tbl[:]])
                tblL = tablesL[par]
                nc.sync.dma_start(out=tblL[:], in_=tbl[:])
                if debug_stop == "ag1":
                    state["step"] = s + 1
                    return

                for ci, (t0, nt) in enumerate(chunk_tiles):
                    pm = chunks_meta[ci]
                    L = pm["L"]
                    idx_t = ip.tile([P, nt * L], I32, tag="idx")
                    nc.sync.dma_start(
                        out=idx_t[:, :nt * L],
                        in_=idxs_d[:, pm["off"]:pm["off"] + nt * L])
                    ssum = sp.tile([P, nt * HID], F32, tag="ssum")
                    tmp = sp.tile([P, nt * HID], F32, tag="stmp")
                    nc.vector.memset(ssum[:, :nt * HID], 0.0)
                    idxc = ip.tile([P, nt], I32, tag="idxc")
                    g8 = gp.tile([P, nt * HID], F32, tag="gbuf")
                    with tc.For_i(0, L, 1) as j:
                        for t in range(nt):
                            nc.vector.tensor_copy(
                                idxc[:, t:t + 1],
                                idx_t[:, t * L:(t + 1) * L][:, bass.ts(j, 1)])
                        for t in range(nt):
                            nc.gpsimd.indirect_dma_start(
                                out=g8[:, t * HID:(t + 1) * HID],
                                out_offset=None,
                                in_=tblL[:],
                                in_offset=bass.IndirectOffsetOnAxis(
                                    ap=idxc[:, t:t + 1], axis=0))
                        nc.vector.tensor_add(ssum[:, :nt * HID],
                                             ssum[:, :nt * HID],
                                             g8[:, :nt * HID])
                    m = ssum[:, :nt * HID]
                    nc.vector.tensor_tensor(out=st3(m), in0=st3(m),
                                            in1=dinv_bc(t0, nt), op=Alu.mult)
                    sl = slice(t0 * HID, (t0 + nt) * HID)
                    into = into_state[:, sl]
                    if mode in ("clenshaw", "clenshaw_first", "clenshaw_last"):
                        cjt = sp.tile([P, nt * HID], F32, tag="cjt")
                        nc.sync.dma_start(
                            out=st3(cjt[:, :nt * HID]),
                            in_=tall0.ap()[t0 * P:(t0 + nt) * P,
                                           cj * HID:(cj + 1) * HID]
                                .rearrange("(t p) d -> p t d", p=P))
                        scl = -1.0 if mode == "clenshaw_last" else -2.0
                        res = sp.tile([P, nt * HID], F32, tag="res")
                        if mode == "clenshaw_first":
                            nc.vector.scalar_tensor_tensor(
                                out=into, in0=m, scalar=scl,
                                in1=cjt[:, :nt * HID],
                                op0=Alu.mult, op1=Alu.add)
                        else:
                            nc.vector.scalar_tensor_tensor(
                                out=tmp[:, :nt * HID], in0=m, scalar=scl,
                                in1=cjt[:, :nt * HID],
                                op0=Alu.mult, op1=Alu.add)
                            nc.vector.tensor_tensor(
                                out=res[:, :nt * HID], in0=tmp[:, :nt * HID],
                                in1=into, op=Alu.subtract)
                            nc.vector.tensor_copy(into, res[:, :nt * HID])
                    elif mode == "fwd_first":
                        nc.vector.tensor_scalar_mul(into, m, -1.0)
                    else:
                        res = sp.tile([P, nt * HID], F32, tag="res")
                        nc.vector.scalar_tensor_tensor(
                            out=res[:, :nt * HID], in0=m, scalar=-2.0,
                            in1=into, op0=Alu.mult, op1=Alu.subtract)
                        nc.vector.tensor_copy(into, res[:, :nt * HID])
                    if write_cin:
                        cst = sp.tile([P, nt * HID], F32, tag="cinst")
                        nc.vector.tensor_tensor(
                            out=st3(cst[:, :nt * HID]), in0=st3(into),
                            in1=dinv_bc(t0, nt), op=Alu.mult)
                        for _b in (0, 1):
                                              nc.sync.dma_start(
                                                  out=lrow_ap(cins[nxt], t0, nt, _b),
                                                  in_=parity_view(cst[:, :nt * HID], nt, _b))
                    if write_craw:
                        crt = sp.tile([P, nt * HID], F16, tag="crawst")
                        nc.vector.tensor_copy(crt[:, :nt * HID], into)
                        for _b in (0, 1):
                                              nc.sync.dma_start(
                                                  out=lrow_ap(craws[nxt], t0, nt, _b),
                                                  in_=parity_view(crt[:, :nt * HID], nt, _b))
                state["step"] = s + 1

            def readback_acc(k, craw, first):
                """acc[f_packed, pair] += W1[k].T @ fp16-transposed craw."""
                np2 = SPC // 2
                craw2 = craw.ap().rearrange("(r two) d -> r (two d)", two=2)
                for c0 in range(0, np2, 512):
                    cw = min(512, np2 - c0)
                    tf = rp.tile([P, 512], F16, tag="tf")
                    nc.sync.dma_start_transpose(out=tf[:, :cw],
                                                in_=craw2[c0:c0 + cw, :])
                    pm = psB.tile([P, 512], F32, space="PSUM", tag="accmm")
                    for h in (0, 64):
                        nc.tensor.matmul(
                            pm[h:h + 64, :cw],
                            lhsT=W1_t[h:h + 64, k * HID:(k + 1) * HID],
                            rhs=tf[h:h + 64, :cw],
                            start=True, stop=True,
                            tile_position=(h, h))
                    if first:
                        nc.vector.tensor_copy(acc[:, c0:c0 + cw], pm[:, :cw])
                    else:
                        nc.vector.tensor_add(acc[:, c0:c0 + cw],
                                             acc[:, c0:c0 + cw], pm[:, :cw])

            def write_h_cin_craw(par):
                """cin/craw <- h (stA), after conv end."""
                for (t0, nt) in chunk_tiles:
                    sl = slice(t0 * HID, (t0 + nt) * HID)
                    cst = sp.tile([P, nt * HID], F32, tag="cinst")
                    nc.vector.tensor_tensor(
                        out=st3(cst[:, :nt * HID]), in0=st3(stA[:, sl]),
                        in1=dinv_bc(t0, nt), op=Alu.mult)
                    for _b in (0, 1):
                                          nc.sync.dma_start(
                                              out=lrow_ap(cins[par], t0, nt, _b),
                                              in_=parity_view(cst[:, :nt * HID], nt, _b))
                    crt = sp.tile([P, nt * HID], F16, tag="crawst")
                    nc.vector.tensor_copy(crt[:, :nt * HID], stA[:, sl])
                    for _b in (0, 1):
                                          nc.sync.dma_start(
                                              out=lrow_ap(craws[par], t0, nt, _b),
                                              in_=parity_view(crt[:, :nt * HID], nt, _b))

            # =============== conv0 (Clenshaw) ===============
            assert K == 5
            stop_after("init", stA)
            if state["go"]:
                gather_step(stB, "clenshaw_first", cj=3)
                stop_after("ag1", stA)
                stop_after("g1", stB)
            if state["go"]:
                gather_step(stA, "clenshaw", cj=2)
                gather_step(stB, "clenshaw", cj=1)
                gather_step(stA, "clenshaw_last", cj=0, write_cin=False)
                stop_after("cheb0", stA)
            if state["go"] and debug_stop == "bndump":
                nc.sync.dma_start(out=dbg_d[:, 0:HID], in_=b0bc[:])
                nc.sync.dma_start(out=dbg_d[:, HID:2 * HID], in_=gsbc[:])
                nc.sync.dma_start(out=dbg_d[:, 2 * HID:3 * HID], in_=gbbc[:])
                prb = sp.tile([P, HID], F32, tag="bnra")
                nc.vector.tensor_add(prb[:], stA[:, 0:HID], b0bc[:])
                nc.sync.dma_start(out=dbg_d[:, 3 * HID:4 * HID], in_=prb[:])
                nc.sync.dma_start(out=dbg_d[:, 4 * HID:5 * HID],
                                  in_=stA[:, 0:HID])
                state["go"] = False

            # BN node-major on stA -> h1 (fully de-aliased: HW DVE ops do
            # not support out aliasing an input)
            for (t0, nt) in (chunk_tiles if state["go"] else []):
                for tt in range(nt):
                    sl1 = slice((t0 + tt) * HID, (t0 + tt + 1) * HID)
                    ra = sp.tile([P, HID], F32, tag="bnra")
                    rb = sp.tile([P, HID], F32, tag="bnrb")
                    nc.vector.tensor_add(ra[:], stA[:, sl1], b0bc[:])
                    nc.vector.tensor_scalar_max(rb[:], ra[:], 0.0)
                    nc.vector.tensor_tensor(out=ra[:], in0=rb[:],
                                            in1=gsbc[:], op=Alu.mult)
                    nc.vector.tensor_add(stA[:, sl1], ra[:], gbbc[:])
            if state["go"]:
                write_h_cin_craw(state["step"] % 2)
                stop_after("conv0", stA)
            if state["go"] and debug_stop == "rb1":
                readback_acc(0, craws[state["step"] % 2], first=True)
                stop_after("rb1", acc)

            # =============== convs 1..NPROP ===============
            for conv in range(NPROP if state["go"] else 0):
                last_conv = conv == NPROP - 1
                readback_acc(0, craws[state["step"] % 2], first=True)
                gather_step(stB, "fwd_first", write_craw=True)
                readback_acc(1, craws[state["step"] % 2], first=False)
                gather_step(stA, "fwd", write_craw=True)
                readback_acc(2, craws[state["step"] % 2], first=False)
                gather_step(stB, "fwd", write_craw=True)
                readback_acc(3, craws[state["step"] % 2], first=False)
                gather_step(stA, "fwd", write_cin=False, write_craw=True)
                readback_acc(4, craws[state["step"] % 2], first=False)
                # BN feature-major on acc (packed [128, SPC//2])
                np2 = SPC // 2
                for c0 in range(0, np2, 512):
                    cw = min(512, np2 - c0)
                    zv = acc[:, c0:c0 + cw]
                    zs = sp.tile([P, 512], F32, tag="bnz")
                    nc.scalar.activation(zs[:, :cw], zv, Act.Relu,
                                         bias=b1p[:, :1])
                    nc.vector.tensor_scalar(
                        out=zv, in0=zs[:, :cw], scalar1=gsp[:, :1],
                        scalar2=gbp[:, :1], op0=Alu.mult, op1=Alu.add)
                # transpose h back to node-major into stA
                for (t0, nt) in chunk_tiles:
                    pt = psA.tile([P, 512], F32, space="PSUM", tag="ptr")
                    ccols = slice((t0 // 2) * P, (t0 // 2) * P + (nt // 2) * P)
                    odd = sp.tile([64, 512], F32, tag="oddh")
                    nc.sync.dma_start(out=odd[:, :(nt // 2) * P],
                                      in_=acc[64:128, ccols])
                    for a in range(nt // 2):
                        cols = slice((t0 // 2 + a) * P, (t0 // 2 + a) * P + P)
                        nc.tensor.transpose(
                            out=pt[:, (2 * a) * HID:(2 * a + 1) * HID],
                            in_=acc[0:64, cols], identity=iden[0:64, 0:64])
                        nc.tensor.transpose(
                            out=pt[:, (2 * a + 1) * HID:(2 * a + 2) * HID],
                            in_=odd[:, a * P:(a + 1) * P],
                            identity=iden[0:64, 0:64])
                    sl = slice(t0 * HID, (t0 + nt) * HID)
                    nc.vector.tensor_copy(stA[:, sl], pt[:, :nt * HID])
                if not last_conv:
                    write_h_cin_craw(state["step"] % 2)
                stop_after(f"conv{conv + 1}", stA)
                if not state["go"]:
                    break

            # =============== pooling + head ===============
            if state["go"]:
                _emit_head = True
            else:
                _emit_head = False
            oneh = None
            if _emit_head:
                oneh = gp.tile([P, TPC * 64], F32, tag="gbuf")
                nc.sync.dma_start(out=oneh[:, :TPC * 64], in_=oneh_d[:])
                pg = psC.tile([64, 64], F32, space="PSUM", tag="pool")
                for t in range(TPC):
                    nc.tensor.matmul(
                        pg[:], lhsT=oneh[:, t * 64:(t + 1) * 64],
                        rhs=stA[:, t * HID:(t + 1) * HID],
                        start=(t == 0), stop=(t == TPC - 1))
                pools = sp.tile([64, HID], F32, tag="pools")
                nc.vector.tensor_copy(pools[:], pg[:])
                nc.sync.dma_start(out=pool_in[:], in_=pools[:])
                nc.gpsimd.collective_compute(
                    "AllReduce", Alu.add,
                    replica_groups=[list(range(NCORES))],
                    ins=[pool_in[:]], outs=[pool_out[:]])
                pooled = sp.tile([64, HID], F32, tag="pooled")
                nc.sync.dma_start(out=pooled[:], in_=pool_out[:])
                invc = sp.tile([64, 1], F32, tag="invc")
                nc.sync.dma_start(out=invc[:], in_=invc_d[:])
                nc.vector.tensor_scalar_mul(pooled[:], pooled[:], invc[:, :1])
                ptp = psC.tile([64, 64], F32, space="PSUM", tag="pool")
                nc.tensor.transpose(out=ptp[:], in_=pooled[:],
                                    identity=iden[0:64, 0:64])
                pooledT = sp.tile([64, HID], F32, tag="pooled")
                nc.vector.tensor_copy(pooledT[:], ptp[:])
                W2t = sp.tile([64, OUT_DIM], F32, tag="w2")
                nc.sync.dma_start(out=W2t[:], in_=W2_d[:])
                pout = psC.tile([64, OUT_DIM], F32, space="PSUM", tag="pout")
                nc.tensor.matmul(pout[:], lhsT=pooledT[:], rhs=W2t[:],
                                 start=True, stop=True)
                b2t = sp.tile([64, OUT_DIM], F32, tag="b2")
                nc.sync.dma_start(out=b2t[:], in_=b2bc_d[:])
                outt = sp.tile([64, OUT_DIM], F32, tag="outt")
                nc.vector.tensor_add(outt[:], pout[:], b2t[:])
                nc.sync.dma_start(out=out_d[:], in_=outt[:])

    nc.compile()
    return nc


# ----------------------------------------------------------------------------
# entry point
# ----------------------------------------------------------------------------

def run(inputs, num_graphs=64, num_step_prop=6, trace=False, verbose=False,
        debug_stop=None):
    in_maps, meta = host_prep(num_graphs=num_graphs,
                              num_step_prop=num_step_prop, verbose=verbose,
                              **inputs)
    nc = build_kernel(meta, debug_stop=debug_stop)
    nc.m = get_hw_module(nc.m)
    import time as _time
    # warm-up call compiles (cached NEFF) and runs once; second call times
    # execution + host I/O only
    res = run_bass_kernel_spmd(nc, in_maps, core_ids=list(range(NCORES)),
                               trace=trace)
    t0 = _time.time()
    res = run_bass_kernel_spmd(nc, in_maps, core_ids=list(range(NCORES)),
                               trace=trace)
    res.exec_time_ns = res.exec_time_ns or int((_time.time() - t0) * 1e9)
    if debug_stop is not None:
        return [r["dbg"] for r in res.results], meta, res
    out = res.results[0]["out"][:num_graphs]
    return out, res


def kernel(**inputs):
    out, _ = run(inputs)
    return out

